# revision 57
# baseline (speedup 1.0000x reference)
"""Trainium2 Bass kernel for a single attention head (v10: no
collectives, fp8 DoubleRow scores, flat i-quarter pipeline).

reference computation (fp32):
    q = query @ Wq + bq ; k = key @ Wk + bk ; v = value @ Wv + bv
    out = softmax((q @ k^T) / 8) @ v

Sharding: 8 cores, core c -> (batch b = c//2, query-half h = c%2). Each core
loads its q half transposed [512, 2048] plus the FULL k^T/v^T of its batch
[512, 4096] -- all host-pre-transposed and host-cast to bf16 (pure layout
prep; all projections/attention FLOPs stay on device). 10 MiB per core, no
inter-core exchange at all (the v2 pair-AllGather design lost ~50 us to
collective launch latency), and no PE input transposes (x^T arrives in the
contraction-major layout the projection matmuls want).

bk is dropped entirely: softmax is invariant to per-query constants.

Per-core dataflow (fp32 PSUM accumulation throughout):
  - x^T loads in 512-col chunks on three DMA queues (SWDGE: k0-3 + v +
    q2-3 in PE-consumption order, Act: k4-7, SP: weights + q0-1); the x^T
    staging pool is closed right after the last projection so back-to-back
    invocations overlap their loads with the previous call's attention
    (chain marginal cost ~83us/body vs ~130us serialized)
  - projections (bf16): lhsT = W [c-chunk, d], rhs = x^T -> Qp^T/Kp^T
    [64, s] quantized to fp8e4 (qp8/kx8) by the sinks; V^T projected then
    PE-transposed to natural bf16 [keys, 66] with col 64 = ones (softmax
    denominator via the PV matmul), col 65 zero pad
  - scores^T tiles [128, 512] via fp8 DoubleRow matmuls (0.5 cyc/row):
    lhsT = kx8[:, :, chunk] [64, 2, 128] whose second k-tile is zeroed
    per-chunk in the k sink, rhs = qp8 broadcast over both k-tiles with a
    stride-0 dim (so tile 1 contributes w1^T q = 0); exp fused with the
    1/8 scale, split between ScalarE (exact exp, ~56%) and DVE
    (Schraudolph int16 bit-trick), assignment varying per (chunk,
    i-quarter) so the bit-trick's sawtooth error decorrelates
  - PV (bf16 -- fp8 P/V fails the 2e-2 gate): lhsT = v[chunk] [128, 66],
    rhs = P^T, accumulated in PSUM -> out^T [66, 512] per i-quarter
    (row 64 = denominator) over all 32 key chunks
  - the whole attention runs as ONE flat 64-step (i-quarter, j-pair)
    pipeline: po accumulators are a single PSUM bank each and
    double-buffered, so the LAG=16-deep scores->exp->PV pipeline crosses
    every i-quarter boundary without draining; epilogues (Act-copy to
    SBUF, PE-transpose, reciprocal+scale, DMA out) are emitted two steps
    after their accumulator closes so their transposes never wait at the
    PE queue head.
"""

import sys

if "/opt/trn_rl_repo" not in sys.path:
    sys.path.insert(0, "/opt/trn_rl_repo")

from contextlib import ExitStack

import numpy as np
import ml_dtypes

import concourse.bass as bass
import concourse.tile as tile
from concourse import bacc, mybir
from concourse.bass_utils import run_bass_kernel_spmd
from concourse.masks import make_identity

F32 = mybir.dt.float32
F32R = mybir.dt.float32r
BF = mybir.dt.bfloat16
FP8 = mybir.dt.float8e4
DR = mybir.MatmulPerfMode.DoubleRow
BF_NP = ml_dtypes.bfloat16
B, S, C, D = 4, 4096, 512, 64
D2 = D + 2          # v padded with [ones, zeros] cols
N_CORES = 8
SQ = S // 2          # query rows per core
SK = S               # key rows per core (full batch)
NJ = SK // 128       # 32 key chunks of 128 rows
NP = NJ // 2         # 16 j-pairs
IQ = SQ // 4         # 512: i-quarter processed per PSUM residency
EXP = mybir.ActivationFunctionType.Exp
CPY = mybir.ActivationFunctionType.Copy
MUL = mybir.AluOpType.mult
ADD = mybir.AluOpType.add

_CACHE = {}

# Schraudolph bf16 exp on DVE: bits(exp(s/8)) ~= round(s*A + B) as int16,
# reinterpreted as bf16 (7 mantissa bits, bias 127). A = 2^7*log2(e)/8;
# B = 127*2^7 - 0.045*2^7 centers the piecewise-linear-mantissa error
# (~+-3% max on the weights; softmax averaging over ~2k keys shrinks it
# far below budget).
SCH_A = 128.0 * 1.4426950408889634 / 8.0
SCH_B = 127.0 * 128.0 - 0.045 * 128.0
I16 = mybir.dt.int16


def _emit(nc, tc, aps):
    qt_d, kt_d, vt_d, wq_d, wk_d, wvp_d, bq_d, bvp_d, out_d = aps

    ctx = ExitStack()
    const = ctx.enter_context(tc.tile_pool(name="const", bufs=1))
    persist = ctx.enter_context(tc.tile_pool(name="persist", bufs=1))
    pt_p = ctx.enter_context(tc.tile_pool(name="pt", bufs=48))
    ep_p = ctx.enter_context(tc.tile_pool(name="ep", bufs=2))
    small_p = ctx.enter_context(tc.tile_pool(name="small", bufs=4))
    out_p = ctx.enter_context(tc.tile_pool(name="outp", bufs=2))
    st_ps = ctx.enter_context(tc.tile_pool(name="stps", bufs=4, space="PSUM"))
    po_ps = ctx.enter_context(tc.tile_pool(name="pops", bufs=1, space="PSUM"))
    ms_ps = ctx.enter_context(tc.tile_pool(name="msps", bufs=2, space="PSUM"))

    ident32 = const.tile([128, 128], F32)
    make_identity(nc, ident32[:])
    identb = const.tile([128, 128], BF)
    nc.vector.tensor_copy(identb[:], ident32[:])
    identr = const.tile([128, 128], F32R)
    nc.vector.tensor_copy(identr[:], ident32[:])

    # weights host-cast to bf16 (layout prep) -> direct load, no cast chain
    # in the first projection's critical path
    wk_sb = const.tile([128, 4, D], BF)
    nc.sync.dma_start(wk_sb[:], wk_d.rearrange("(cc p) d -> p cc d", p=128))
    wq_sb = const.tile([128, 4, D], BF)
    nc.sync.dma_start(wq_sb[:], wq_d.rearrange("(cc p) d -> p cc d", p=128))
    wvp_sb = const.tile([128, 4, D2], BF)
    nc.sync.dma_start(wvp_sb[:], wvp_d.rearrange("(cc p) d -> p cc d", p=128))
    bq_sb = const.tile([D, 1], F32)
    nc.sync.dma_start(bq_sb[:], bq_d[:])
    bvp_sb = const.tile([D2, 1], F32)
    nc.sync.dma_start(bvp_sb[:], bvp_d[:])

    # x^T staging lives in its own pool, closed right after the last
    # projection is emitted: its 80 KiB/partition becomes free for the NEXT
    # kernel invocation's loads, so back-to-back calls overlap in SBUF
    # instead of serializing on the staging region (verified: reps=16 chain
    # marginal cost drops ~130us -> ~83us per body, output bit-identical)
    load_ctx = ExitStack()
    load_p = load_ctx.enter_context(tc.tile_pool(name="xload", bufs=1))
    qts = load_p.tile([128, 4, SQ], BF)    # q^T staged (c on partitions)
    kts = load_p.tile([128, 4, SK], BF)    # k^T staged
    vts = load_p.tile([128, 4, SK], BF)    # v^T staged
    # fp8 Qp^T / Kp^T for DoubleRow scores matmuls (0.5 cyc/row). kx8's
    # second k-tile is zeroed per-chunk by sink_k; the rhs broadcasts Qp^T
    # over both k-tiles with a stride-0 dim, so tile 1 contributes 0.
    qp8 = persist.tile([D, SQ], FP8)
    kx8 = persist.tile([D, 2, SK], FP8)
    vx = persist.tile([128, NJ, D2], BF)   # v natural + ones col

    # ---- staged loads: 512-col chunks, dependency-ordered ---------------
    ktv = kt_d.rearrange("(cc p) s -> p cc s", p=128)
    qtv = qt_d.rearrange("(cc p) s -> p cc s", p=128)
    vtv = vt_d.rearrange("(cc p) s -> p cc s", p=128)

    def ch(x, g, w=512):
        return x[:, :, g * w : (g + 1) * w]

    # SWDGE gets everything the PE needs early plus the mid-stream v/q
    # chunks, in exactly the order the in-order PE stream consumes them;
    # its queue drains by ~25us, so in back-to-back invocations the NEXT
    # call's SWDGE loads prefetch into the freed staging region while this
    # call's attention still runs. k4-7 ride the Act queue (landing ~5us,
    # needed at 10us+); SP carries weights + the two front q chunks.
    for g in range(4, 8):
        nc.scalar.dma_start(ch(kts, g), ch(ktv, g))
    nc.sync.dma_start(ch(qts, 0), ch(qtv, 0))
    nc.sync.dma_start(ch(qts, 1), ch(qtv, 1))
    for t, g in [("k", 0), ("k", 1), ("k", 2), ("v", 0), ("k", 3), ("v", 1),
                 ("v", 2), ("v", 3), ("v", 4), ("v", 5), ("q", 2), ("q", 3),
                 ("v", 6), ("v", 7)]:
        if t == "k":
            nc.gpsimd.dma_start(ch(kts, g), ch(ktv, g))
        elif t == "q":
            nc.gpsimd.dma_start(ch(qts, g), ch(qtv, g))
        else:
            nc.gpsimd.dma_start(ch(vts, g), ch(vtv, g))

    # ---- projections ----------------------------------------------------
    def proj(xts, w_sb, m, g, sink):
        """Project one 512-col group: pp [m, 512] PSUM; sink consumes it."""
        pp = ms_ps.tile([D2, 512], F32, tag="ms")
        for cc in range(4):
            nc.tensor.matmul(
                pp[:m, :], w_sb[:, cc, :m], xts[:, cc, g * 512 : (g + 1) * 512],
                start=(cc == 0), stop=(cc == 3),
            )
        sink(pp)

    # bias-free PSUM drains run on ScalarE (Act Copy needs no act table, so
    # no table thrash with Exp); biased sinks stay on DVE
    def sink_q(g):
        def f(pp):
            sl = slice(g * 512, (g + 1) * 512)
            nc.vector.tensor_scalar_add(qp8[:, sl], pp[:D, :], bq_sb[:])
        return f

    def sink_k(g):
        # pp [64, 512] = key rows g*512..(g+1)*512 (no bias: bk dropped);
        # the chunk's zero k-tile is filled here too (spread across Act's
        # idle early phase rather than one big serial memset)
        def f(pp):
            sl = slice(g * 512, (g + 1) * 512)
            nc.scalar.activation(kx8[:, 0, sl], pp[:D, :], CPY)
            nc.scalar.memzero(kx8[:, 1, sl])
        return f

    def sink_v(g):
        def f(pp):
            vt = ep_p.tile([D2, 512], BF, tag="vt")
            nc.vector.tensor_scalar_add(vt[:], pp[:, :], bvp_sb[:])
            # 4 transposes batched into one PSUM tile -> single drain
            vnp = ms_ps.tile([128, 4, D2], BF, tag="ms")
            for r in range(4):
                nc.tensor.transpose(
                    vnp[:, r, :], vt[:, r * 128 : (r + 1) * 128], identb[:D2, :D2]
                )
            nc.vector.tensor_copy(vx[:, g * 4 : g * 4 + 4, :], vnp[:])
        return f

    # ---- attention helpers ----------------------------------------------
    def scores_exp(step, lp, iq):
        """Scores + exp for one j-pair against i-quarter iq; returns 2 P^T
        bf16 APs ([128, 512] each, one per kx8 half). The exp stream
        alternates between ScalarE (exact exp) and DVE (Schraudolph)."""
        sts = []
        for half in range(2):
            j = 2 * lp + half
            st = st_ps.tile([128, 512], F32, tag="st")
            nc.tensor.matmul(
                st[:],
                kx8[:, :, j * 128 : (j + 1) * 128],
                qp8[:, iq * IQ : (iq + 1) * IQ]
                .unsqueeze(1).broadcast_to([D, 2, IQ]),
                perf_mode=DR,
            )
            if (step + half) % 2 == 0 or (step % 8 == 3 and half == 1):
                pt = pt_p.tile([128, 512], BF, tag="pt")
                nc.scalar.activation(pt[:], st[:], EXP, scale=0.125)
                sts.append(pt[:])
            else:
                pt16 = pt_p.tile([128, 512], I16, tag="pt")
                nc.vector.tensor_scalar(pt16[:], st[:], SCH_A, SCH_B, MUL, ADD)
                sts.append(pt16[:].bitcast(BF))
        return sts

    def pv(lp, po, sts):
        for half in range(2):
            nc.tensor.matmul(
                po[:],
                vx[:, 2 * lp + half, :],
                sts[half],
                start=(lp == 0 and half == 0),
                stop=(lp == NP - 1 and half == 1),
            )

    def epilogue(iq, po):
        ot = ep_p.tile([D2, IQ], F32R, tag="ot")
        nc.scalar.activation(ot[:], po[:], CPY)
        osb = out_p.tile([128, 4, D], F32, tag="osb")
        onat = ms_ps.tile([128, 4, D2], F32R, tag="ms")
        for r in range(4):
            nc.tensor.transpose(
                onat[:, r, :], ot[:, r * 128 : (r + 1) * 128], identr[:D2, :D2]
            )
        for r in range(4):
            rs = small_p.tile([128, 1], F32, tag="rs")
            nc.vector.reciprocal(rs[:], onat[:, r, D : D + 1])
            nc.vector.tensor_scalar_mul(osb[:, r, :], onat[:, r, :D], rs[:])
        nc.sync.dma_start(
            out_d[iq * IQ : (iq + 1) * IQ, :].rearrange("(t p) d -> p t d", p=128),
            osb[:],
        )

    # ---- schedule -------------------------------------------------------
    # One flat 64-step pipeline over (i-quarter, j-pair): po is a single
    # PSUM bank per i-quarter and double-buffered, so the LAG-deep
    # scores->exp->PV pipeline runs continuously across every i-quarter
    # boundary with no drain bubble. The k/q/v projections interleave into
    # the first steps just ahead of their consumers; each epilogue is
    # emitted two steps after its accumulator closes so its PE transposes
    # never sit at the queue head waiting for the Act copy.
    LAG = 16
    proj(kts, wk_sb, D, 0, sink_k(0))
    proj(kts, wk_sb, D, 1, sink_k(1))
    proj(qts, wq_sb, D, 0, sink_q(0))
    proj(qts, wq_sb, D, 1, sink_q(1))

    NIQ = SQ // IQ
    po_of = {}
    pend = []
    epi_q = []

    def pop_one(step):
        iq0, lp0, s0 = pend.pop(0)
        if lp0 == 0:
            po_of[iq0] = po_ps.tile([D2, IQ], F32, tag="po", name="po", bufs=2)
        pv(lp0, po_of[iq0], s0)
        if lp0 == NP - 1:
            epi_q.append((step + 2, iq0, po_of.pop(iq0)))

    for step in range(NIQ * NP):
        iq, lp = divmod(step, NP)
        if step % 2 == 0 and step < 16:
            g = step // 2
            if g + 2 < 8:
                proj(kts, wk_sb, D, g + 2, sink_k(g + 2))
            proj(vts, wvp_sb, D2, g, sink_v(g))
        elif step in (17, 19):
            proj(qts, wq_sb, D, (step - 13) // 2, sink_q((step - 13) // 2))
        elif step == 20:
            # all projections are emitted -> release the staging region so
            # the next invocation's loads can prefetch into it
            load_ctx.close()
        pend.append((iq, lp, scores_exp(step, lp, iq)))
        if len(pend) > LAG:
            pop_one(step)
        while epi_q and epi_q[0][0] <= step:
            _, iq0, po0 = epi_q.pop(0)
            epilogue(iq0, po0)
    vstep = NIQ * NP
    while pend:
        pop_one(vstep)
        while epi_q and epi_q[0][0] <= vstep:
            _, iq0, po0 = epi_q.pop(0)
            epilogue(iq0, po0)
        vstep += 1
    for _, iq0, po0 in epi_q:
        epilogue(iq0, po0)
    ctx.close()


def _build(reps=1):
    nc = bacc.Bacc("TRN2", target_bir_lowering=False, debug=False, num_devices=N_CORES)
    aps = (
        nc.dram_tensor("qt", [C, SQ], BF, kind="ExternalInput").ap(),
        nc.dram_tensor("kt", [C, SK], BF, kind="ExternalInput").ap(),
        nc.dram_tensor("vt", [C, SK], BF, kind="ExternalInput").ap(),
        nc.dram_tensor("wq", [C, D], BF, kind="ExternalInput").ap(),
        nc.dram_tensor("wk", [C, D], BF, kind="ExternalInput").ap(),
        nc.dram_tensor("wvp", [C, D2], BF, kind="ExternalInput").ap(),
        nc.dram_tensor("bq", [D, 1], F32, kind="ExternalInput").ap(),
        nc.dram_tensor("bvp", [D2, 1], F32, kind="ExternalInput").ap(),
        nc.dram_tensor("out", [SQ, D], F32, kind="ExternalOutput").ap(),
    )
    with tile.TileContext(nc) as tc:
        for _ in range(reps):
            _emit(nc, tc, aps)
    nc.compile()
    return nc


def get_nc():
    if "nc" not in _CACHE:
        _CACHE["nc"] = _build()
    return _CACHE["nc"]


def make_in_maps(query, key_, value, Wq, bq, Wk, bk, Wv, bv):
    query, key_, value, Wq, bq, Wk, bk, Wv, bv = (
        np.asarray(a, dtype=np.float32)
        for a in (query, key_, value, Wq, bq, Wk, bk, Wv, bv)
    )
    wvp = np.concatenate([Wv, np.zeros((C, 2), np.float32)], axis=1)
    bvp = np.concatenate([bv, np.asarray([1.0, 0.0], np.float32)])[:, None]
    shared = {
        "wq": np.ascontiguousarray(Wq.astype(BF_NP)),
        "wk": np.ascontiguousarray(Wk.astype(BF_NP)),
        "wvp": np.ascontiguousarray(wvp.astype(BF_NP)),
        "bq": np.ascontiguousarray(bq[:, None]),
        "bvp": np.ascontiguousarray(bvp),
    }
    # host-side layout prep (cast + transpose only): k^T/v^T once per batch,
    # shared by the two cores that split the batch's queries
    ktb = [np.ascontiguousarray(key_[b].astype(BF_NP).T) for b in range(B)]
    vtb = [np.ascontiguousarray(value[b].astype(BF_NP).T) for b in range(B)]
    in_maps = []
    for c in range(N_CORES):
        b, h = divmod(c, 2)
        sl = slice(h * SQ, (h + 1) * SQ)
        in_maps.append(
            {
                "qt": np.ascontiguousarray(query[b, sl, :].astype(BF_NP).T),
                "kt": ktb[b],
                "vt": vtb[b],
                **shared,
            }
        )
    return in_maps


def assemble(results):
    out = np.empty((B, S, D), np.float32)
    for c in range(N_CORES):
        b, h = divmod(c, 2)
        out[b, h * SQ : (h + 1) * SQ, :] = results[c]["out"]
    return out


def kernel(query=None, key_=None, value=None, Wq=None, bq=None, Wk=None,
           bk=None, Wv=None, bv=None, key=None, **_):
    if key_ is None:
        key_ = key          # spec names this input "key"; reference uses "key_"
    nc = get_nc()
    in_maps = make_in_maps(query, key_, value, Wq, bq, Wk, bk, Wv, bv)
    res = run_bass_kernel_spmd(nc, in_maps, list(range(N_CORES)))
    return assemble(res.results)


# revision 58
# speedup vs baseline: 2.8679x; 2.8679x over previous
"""Trainium2 Bass kernel for a single attention head (v10: no
collectives, fp8 DoubleRow scores, flat i-quarter pipeline).

reference computation (fp32):
    q = query @ Wq + bq ; k = key @ Wk + bk ; v = value @ Wv + bv
    out = softmax((q @ k^T) / 8) @ v

Sharding: 8 cores, core c -> (batch b = c//2, query-half h = c%2). Each core
loads its q half transposed [512, 2048] plus the FULL k^T/v^T of its batch
[512, 4096] -- all host-pre-transposed and host-cast to bf16 (pure layout
prep; all projections/attention FLOPs stay on device). 10 MiB per core, no
inter-core exchange at all (the v2 pair-AllGather design lost ~50 us to
collective launch latency), and no PE input transposes (x^T arrives in the
contraction-major layout the projection matmuls want).

bk is dropped entirely: softmax is invariant to per-query constants.

Per-core dataflow (fp32 PSUM accumulation throughout):
  - x^T loads in 512-col chunks on three DMA queues (SWDGE: k0-3 + v +
    q2-3 in PE-consumption order, Act: k4-7, SP: weights + q0-1); the x^T
    staging pool is closed right after the last projection so back-to-back
    invocations overlap their loads with the previous call's attention
    (chain marginal cost ~83us/body vs ~130us serialized)
  - projections (bf16): lhsT = W [c-chunk, d], rhs = x^T -> Qp^T/Kp^T
    [64, s] quantized to fp8e4 (qp8/kx8) by the sinks; V^T projected then
    PE-transposed to natural bf16 [keys, 66] with col 64 = ones (softmax
    denominator via the PV matmul), col 65 zero pad
  - scores^T tiles [128, 512] via fp8 DoubleRow matmuls (0.5 cyc/row):
    lhsT = kx8[:, :, chunk] [64, 2, 128] whose second k-tile is zeroed
    per-chunk in the k sink, rhs = qp8 broadcast over both k-tiles with a
    stride-0 dim (so tile 1 contributes w1^T q = 0); exp fused with the
    1/8 scale, split between ScalarE (exact exp, ~56%) and DVE
    (Schraudolph int16 bit-trick), assignment varying per (chunk,
    i-quarter) so the bit-trick's sawtooth error decorrelates
  - PV (bf16 -- fp8 P/V fails the 2e-2 gate): lhsT = v[chunk] [128, 66],
    rhs = P^T, accumulated in PSUM -> out^T [66, 512] per i-quarter
    (row 64 = denominator) over all 32 key chunks
  - the whole attention runs as ONE flat 64-step (i-quarter, j-pair)
    pipeline: po accumulators are a single PSUM bank each and
    double-buffered, so the LAG=16-deep scores->exp->PV pipeline crosses
    every i-quarter boundary without draining; epilogues (Act-copy to
    SBUF, PE-transpose, reciprocal+scale, DMA out) are emitted two steps
    after their accumulator closes so their transposes never wait at the
    PE queue head.
"""

import sys

if "/opt/trn_rl_repo" not in sys.path:
    sys.path.insert(0, "/opt/trn_rl_repo")

from contextlib import ExitStack

import numpy as np
import ml_dtypes

import concourse.bass as bass
import concourse.tile as tile
from concourse import bacc, mybir
from concourse.bass_utils import run_bass_kernel_spmd
from concourse.masks import make_identity

F32 = mybir.dt.float32
F32R = mybir.dt.float32r
BF = mybir.dt.bfloat16
FP8 = mybir.dt.float8e4
DR = mybir.MatmulPerfMode.DoubleRow
BF_NP = ml_dtypes.bfloat16
F8_NP = ml_dtypes.float8_e4m3fn
B, S, C, D = 4, 4096, 512, 64
D2 = D + 2          # v padded with [ones, zeros] cols
N_CORES = 8
SQ = S // 2          # query rows per core
SK = S               # key rows per core (full batch)
NJ = SK // 128       # 32 key chunks of 128 rows
NP = NJ // 2         # 16 j-pairs
IQ = SQ // 4         # 512: i-quarter processed per PSUM residency
EXP = mybir.ActivationFunctionType.Exp
CPY = mybir.ActivationFunctionType.Copy
MUL = mybir.AluOpType.mult
ADD = mybir.AluOpType.add

_CACHE = {}

# Schraudolph bf16 exp on DVE: bits(exp(s/8)) ~= round(s*A + B) as int16,
# reinterpreted as bf16 (7 mantissa bits, bias 127). A = 2^7*log2(e)/8;
# B = 127*2^7 - 0.045*2^7 centers the piecewise-linear-mantissa error
# (~+-3% max on the weights; softmax averaging over ~2k keys shrinks it
# far below budget).
SCH_A = 128.0 * 1.4426950408889634 / 8.0
SCH_B = 127.0 * 128.0 - 0.045 * 128.0
I16 = mybir.dt.int16


def _emit(nc, tc, aps):
    qt_d, kt_d, vt_d, wq_d, wk_d, wvp_d, bq_d, bvp_d, out_d = aps

    ctx = ExitStack()
    const = ctx.enter_context(tc.tile_pool(name="const", bufs=1))
    persist = ctx.enter_context(tc.tile_pool(name="persist", bufs=1))
    pt_p = ctx.enter_context(tc.tile_pool(name="pt", bufs=48))
    ep_p = ctx.enter_context(tc.tile_pool(name="ep", bufs=2))
    small_p = ctx.enter_context(tc.tile_pool(name="small", bufs=4))
    out_p = ctx.enter_context(tc.tile_pool(name="outp", bufs=2))
    st_ps = ctx.enter_context(tc.tile_pool(name="stps", bufs=4, space="PSUM"))
    po_ps = ctx.enter_context(tc.tile_pool(name="pops", bufs=1, space="PSUM"))
    ms_ps = ctx.enter_context(tc.tile_pool(name="msps", bufs=2, space="PSUM"))

    ident32 = const.tile([128, 128], F32)
    make_identity(nc, ident32[:])
    identb = const.tile([128, 128], BF)
    nc.vector.tensor_copy(identb[:], ident32[:])
    identr = const.tile([128, 128], F32R)
    nc.vector.tensor_copy(identr[:], ident32[:])

    # weights host-cast to bf16 (layout prep) -> direct load, no cast chain
    # in the first projection's critical path
    wk_sb = const.tile([128, 4, 2, D], FP8)
    nc.sync.dma_start(wk_sb[:], wk_d.rearrange("(cc p) t d -> p cc t d", p=128))
    wq_sb = const.tile([128, 4, 2, D], FP8)
    nc.sync.dma_start(wq_sb[:], wq_d.rearrange("(cc p) t d -> p cc t d", p=128))
    wvp_sb = const.tile([128, 4, D2], BF)
    nc.sync.dma_start(wvp_sb[:], wvp_d.rearrange("(cc p) d -> p cc d", p=128))
    bq_sb = const.tile([D, 1], F32)
    nc.sync.dma_start(bq_sb[:], bq_d[:])
    bvp_sb = const.tile([D2, 1], F32)
    nc.sync.dma_start(bvp_sb[:], bvp_d[:])

    # x^T staging lives in its own pool, closed right after the last
    # projection is emitted: its 80 KiB/partition becomes free for the NEXT
    # kernel invocation's loads, so back-to-back calls overlap in SBUF
    # instead of serializing on the staging region (verified: reps=16 chain
    # marginal cost drops ~130us -> ~83us per body, output bit-identical)
    load_ctx = ExitStack()
    load_p = load_ctx.enter_context(tc.tile_pool(name="xload", bufs=1))
    qts = load_p.tile([128, 4, SQ], FP8)   # q^T staged fp8 (c on partitions)
    kts = load_p.tile([128, 4, SK], FP8)   # k^T staged fp8
    vts = load_p.tile([128, 4, SK], BF)    # v^T staged
    # fp8 Qp^T / Kp^T for DoubleRow scores matmuls (0.5 cyc/row). kx8's
    # second k-tile is zeroed per-chunk by sink_k; the rhs broadcasts Qp^T
    # over both k-tiles with a stride-0 dim, so tile 1 contributes 0.
    qp8 = persist.tile([D, SQ], FP8)
    kx8 = persist.tile([D, 2, SK], FP8)
    vx = persist.tile([128, NJ, D2], BF)   # v natural + ones col

    # ---- staged loads: 512-col chunks, dependency-ordered ---------------
    ktv = kt_d.rearrange("(cc p) s -> p cc s", p=128)
    qtv = qt_d.rearrange("(cc p) s -> p cc s", p=128)
    vtv = vt_d.rearrange("(cc p) s -> p cc s", p=128)

    def ch(x, g, w=512):
        return x[:, :, g * w : (g + 1) * w]

    # SWDGE gets everything the PE needs early plus the mid-stream v/q
    # chunks, in exactly the order the in-order PE stream consumes them;
    # its queue drains by ~25us, so in back-to-back invocations the NEXT
    # call's SWDGE loads prefetch into the freed staging region while this
    # call's attention still runs. k4-7 ride the Act queue (landing ~5us,
    # needed at 10us+); SP carries weights + the two front q chunks.
    for g in range(4, 8):
        nc.scalar.dma_start(ch(kts, g), ch(ktv, g))
    nc.sync.dma_start(ch(qts, 0), ch(qtv, 0))
    nc.sync.dma_start(ch(qts, 1), ch(qtv, 1))
    for t, g in [("k", 0), ("k", 1), ("k", 2), ("v", 0), ("k", 3), ("v", 1),
                 ("v", 2), ("v", 3), ("v", 4), ("v", 5), ("q", 2), ("q", 3),
                 ("v", 6), ("v", 7)]:
        if t == "k":
            nc.gpsimd.dma_start(ch(kts, g), ch(ktv, g))
        elif t == "q":
            nc.gpsimd.dma_start(ch(qts, g), ch(qtv, g))
        else:
            nc.gpsimd.dma_start(ch(vts, g), ch(vtv, g))

    # ---- projections ----------------------------------------------------
    def proj(xts, w_sb, m, g, sink):
        """bf16 projection (v path): pp [m, 512] PSUM; sink consumes it."""
        pp = ms_ps.tile([D2, 512], F32, tag="ms")
        for cc in range(4):
            nc.tensor.matmul(
                pp[:m, :], w_sb[:, cc, :m], xts[:, cc, g * 512 : (g + 1) * 512],
                start=(cc == 0), stop=(cc == 3),
            )
        sink(pp)

    def proj8(xts, wx_sb, g, sink):
        """fp8 DoubleRow projection (q/k): lhsT = (W_hi, W_lo) fp8 pair
        (W_hi + W_lo matches W to ~0.03%, so only the fp8 x quantization
        costs accuracy -- and qp/kp get quantized to fp8 downstream
        anyway); rhs broadcasts the fp8 x chunk over both k-tiles.
        0.5 cyc/row: halves the q/k projection PE time."""
        pp = ms_ps.tile([D2, 512], F32, tag="ms")
        for cc in range(4):
            nc.tensor.matmul(
                pp[:D, :], wx_sb[:, cc, :, :],
                xts[:, cc, g * 512 : (g + 1) * 512]
                .unsqueeze(1).broadcast_to([128, 2, 512]),
                start=(cc == 0), stop=(cc == 3), perf_mode=DR,
            )
        sink(pp)

    # bias-free PSUM drains run on ScalarE (Act Copy needs no act table, so
    # no table thrash with Exp); biased sinks stay on DVE
    def sink_q(g):
        def f(pp):
            sl = slice(g * 512, (g + 1) * 512)
            nc.vector.tensor_scalar_add(qp8[:, sl], pp[:D, :], bq_sb[:])
        return f

    def sink_k(g):
        # pp [64, 512] = key rows g*512..(g+1)*512 (no bias: bk dropped);
        # the chunk's zero k-tile is filled here too (spread across Act's
        # idle early phase rather than one big serial memset)
        def f(pp):
            sl = slice(g * 512, (g + 1) * 512)
            nc.scalar.activation(kx8[:, 0, sl], pp[:D, :], CPY)
            nc.scalar.memzero(kx8[:, 1, sl])
        return f

    def sink_v(g):
        def f(pp):
            vt = ep_p.tile([D2, 512], BF, tag="vt")
            nc.vector.tensor_scalar_add(vt[:], pp[:, :], bvp_sb[:])
            # 4 transposes batched into one PSUM tile -> single drain
            vnp = ms_ps.tile([128, 4, D2], BF, tag="ms")
            for r in range(4):
                nc.tensor.transpose(
                    vnp[:, r, :], vt[:, r * 128 : (r + 1) * 128], identb[:D2, :D2]
                )
            nc.vector.tensor_copy(vx[:, g * 4 : g * 4 + 4, :], vnp[:])
        return f

    # ---- attention helpers ----------------------------------------------
    def scores_exp(step, lp, iq):
        """Scores + exp for one j-pair against i-quarter iq; returns 2 P^T
        bf16 APs ([128, 512] each, one per kx8 half). The exp stream
        alternates between ScalarE (exact exp) and DVE (Schraudolph)."""
        sts = []
        for half in range(2):
            j = 2 * lp + half
            st = st_ps.tile([128, 512], F32, tag="st")
            nc.tensor.matmul(
                st[:],
                kx8[:, :, j * 128 : (j + 1) * 128],
                qp8[:, iq * IQ : (iq + 1) * IQ]
                .unsqueeze(1).broadcast_to([D, 2, IQ]),
                perf_mode=DR,
            )
            if (step + half) % 2 == 0 or (step % 8 == 3 and half == 1):
                pt = pt_p.tile([128, 512], BF, tag="pt")
                nc.scalar.activation(pt[:], st[:], EXP, scale=0.125)
                sts.append(pt[:])
            else:
                pt16 = pt_p.tile([128, 512], I16, tag="pt")
                nc.vector.tensor_scalar(pt16[:], st[:], SCH_A, SCH_B, MUL, ADD)
                sts.append(pt16[:].bitcast(BF))
        return sts

    def pv(lp, po, sts):
        for half in range(2):
            nc.tensor.matmul(
                po[:],
                vx[:, 2 * lp + half, :],
                sts[half],
                start=(lp == 0 and half == 0),
                stop=(lp == NP - 1 and half == 1),
            )

    def epilogue(iq, po):
        ot = ep_p.tile([D2, IQ], F32R, tag="ot")
        nc.scalar.activation(ot[:], po[:], CPY)
        osb = out_p.tile([128, 4, D], F32, tag="osb")
        onat = ms_ps.tile([128, 4, D2], F32R, tag="ms")
        for r in range(4):
            nc.tensor.transpose(
                onat[:, r, :], ot[:, r * 128 : (r + 1) * 128], identr[:D2, :D2]
            )
        for r in range(4):
            rs = small_p.tile([128, 1], F32, tag="rs")
            nc.vector.reciprocal(rs[:], onat[:, r, D : D + 1])
            nc.vector.tensor_scalar_mul(osb[:, r, :], onat[:, r, :D], rs[:])
        nc.sync.dma_start(
            out_d[iq * IQ : (iq + 1) * IQ, :].rearrange("(t p) d -> p t d", p=128),
            osb[:],
        )

    # ---- schedule -------------------------------------------------------
    # One flat 64-step pipeline over (i-quarter, j-pair): po is a single
    # PSUM bank per i-quarter and double-buffered, so the LAG-deep
    # scores->exp->PV pipeline runs continuously across every i-quarter
    # boundary with no drain bubble. The k/q/v projections interleave into
    # the first steps just ahead of their consumers; each epilogue is
    # emitted two steps after its accumulator closes so its PE transposes
    # never sit at the queue head waiting for the Act copy.
    LAG = 16
    proj8(kts, wk_sb, 0, sink_k(0))
    proj8(kts, wk_sb, 1, sink_k(1))
    proj8(qts, wq_sb, 0, sink_q(0))
    proj8(qts, wq_sb, 1, sink_q(1))

    NIQ = SQ // IQ
    po_of = {}
    pend = []
    epi_q = []

    def pop_one(step):
        iq0, lp0, s0 = pend.pop(0)
        if lp0 == 0:
            po_of[iq0] = po_ps.tile([D2, IQ], F32, tag="po", name="po", bufs=2)
        pv(lp0, po_of[iq0], s0)
        if lp0 == NP - 1:
            epi_q.append((step + 2, iq0, po_of.pop(iq0)))

    for step in range(NIQ * NP):
        iq, lp = divmod(step, NP)
        if step % 2 == 0 and step < 16:
            g = step // 2
            if g + 2 < 8:
                proj8(kts, wk_sb, g + 2, sink_k(g + 2))
            proj(vts, wvp_sb, D2, g, sink_v(g))
        elif step in (17, 19):
            proj8(qts, wq_sb, (step - 13) // 2, sink_q((step - 13) // 2))
        elif step == 20:
            # all projections are emitted -> release the staging region so
            # the next invocation's loads can prefetch into it
            load_ctx.close()
        pend.append((iq, lp, scores_exp(step, lp, iq)))
        if len(pend) > LAG:
            pop_one(step)
        while epi_q and epi_q[0][0] <= step:
            _, iq0, po0 = epi_q.pop(0)
            epilogue(iq0, po0)
    vstep = NIQ * NP
    while pend:
        pop_one(vstep)
        while epi_q and epi_q[0][0] <= vstep:
            _, iq0, po0 = epi_q.pop(0)
            epilogue(iq0, po0)
        vstep += 1
    for _, iq0, po0 in epi_q:
        epilogue(iq0, po0)
    ctx.close()


def _build(reps=1):
    nc = bacc.Bacc("TRN2", target_bir_lowering=False, debug=False, num_devices=N_CORES)
    aps = (
        nc.dram_tensor("qt", [C, SQ], FP8, kind="ExternalInput").ap(),
        nc.dram_tensor("kt", [C, SK], FP8, kind="ExternalInput").ap(),
        nc.dram_tensor("vt", [C, SK], BF, kind="ExternalInput").ap(),
        nc.dram_tensor("wq", [C, 2, D], FP8, kind="ExternalInput").ap(),
        nc.dram_tensor("wk", [C, 2, D], FP8, kind="ExternalInput").ap(),
        nc.dram_tensor("wvp", [C, D2], BF, kind="ExternalInput").ap(),
        nc.dram_tensor("bq", [D, 1], F32, kind="ExternalInput").ap(),
        nc.dram_tensor("bvp", [D2, 1], F32, kind="ExternalInput").ap(),
        nc.dram_tensor("out", [SQ, D], F32, kind="ExternalOutput").ap(),
    )
    with tile.TileContext(nc) as tc:
        for _ in range(reps):
            _emit(nc, tc, aps)
    nc.compile()
    return nc


def get_nc():
    if "nc" not in _CACHE:
        _CACHE["nc"] = _build()
    return _CACHE["nc"]


def make_in_maps(query, key_, value, Wq, bq, Wk, bk, Wv, bv):
    query, key_, value, Wq, bq, Wk, bk, Wv, bv = (
        np.asarray(a, dtype=np.float32)
        for a in (query, key_, value, Wq, bq, Wk, bk, Wv, bv)
    )
    wvp = np.concatenate([Wv, np.zeros((C, 2), np.float32)], axis=1)
    bvp = np.concatenate([bv, np.asarray([1.0, 0.0], np.float32)])[:, None]
    def hilo(w):
        hi = w.astype(F8_NP)
        lo = (w - hi.astype(np.float32)).astype(F8_NP)
        return np.ascontiguousarray(np.stack([hi, lo], axis=1))

    shared = {
        "wq": hilo(Wq),
        "wk": hilo(Wk),
        "wvp": np.ascontiguousarray(wvp.astype(BF_NP)),
        "bq": np.ascontiguousarray(bq[:, None]),
        "bvp": np.ascontiguousarray(bvp),
    }
    # host-side layout prep (cast + transpose only): k^T/v^T once per batch,
    # shared by the two cores that split the batch's queries
    ktb = [np.ascontiguousarray(key_[b].astype(F8_NP).T) for b in range(B)]
    vtb = [np.ascontiguousarray(value[b].astype(BF_NP).T) for b in range(B)]
    in_maps = []
    for c in range(N_CORES):
        b, h = divmod(c, 2)
        sl = slice(h * SQ, (h + 1) * SQ)
        in_maps.append(
            {
                "qt": np.ascontiguousarray(query[b, sl, :].astype(F8_NP).T),
                "kt": ktb[b],
                "vt": vtb[b],
                **shared,
            }
        )
    return in_maps


def assemble(results):
    out = np.empty((B, S, D), np.float32)
    for c in range(N_CORES):
        b, h = divmod(c, 2)
        out[b, h * SQ : (h + 1) * SQ, :] = results[c]["out"]
    return out


def kernel(query=None, key_=None, value=None, Wq=None, bq=None, Wk=None,
           bk=None, Wv=None, bv=None, key=None, **_):
    if key_ is None:
        key_ = key          # spec names this input "key"; reference uses "key_"
    nc = get_nc()
    in_maps = make_in_maps(query, key_, value, Wq, bq, Wk, bk, Wv, bv)
    res = run_bass_kernel_spmd(nc, in_maps, list(range(N_CORES)))
    return assemble(res.results)
